# revision 12
# baseline (speedup 1.0000x reference)
"""TabNet DecisionStep kernel for 8 trn2 NeuronCores (data-parallel batch shard).

Full inputs in, full outputs out. Internally:
  - batch 8192 sharded 8x1024; weights replicated (fp32r = tf32-like m11).
  - attentive transformer: two-pass z1 matmul; BN0 stats via AllReduce; BN0
    affine (+uniform prior) folded into a rescaled second matmul (Wa'').
  - sparsemax via top-16 (max8/match_replace) closed form (support <= 12).
  - feature transformer: 3 GLU stages in transposed layout (features on
    partitions); per-stage BN stats AllReduce; linear biases dropped
    (BN shift-invariant); sqrt(1/2) residual scales folded into W2 and
    g2/bt2 host-side.
"""
import numpy as np
import concourse.bass as bass
import concourse.bacc as bacc
import concourse.mybir as mybir
from concourse import tile
from concourse.bass_utils import run_bass_kernel_spmd
from concourse.masks import make_identity

F32 = mybir.dt.float32
F32R = mybir.dt.float32r
AF = mybir.ActivationFunctionType
OP = mybir.AluOpType

N_CORES = 8
CORE_IDS = list(range(N_CORES))
B, IN_DIM, FEAT, HALF = 8192, 2048, 1024, 512
BS = B // N_CORES            # 1024 rows per core
NCH = BS // 128              # 8 chunks of 128 rows
EPS = 1e-5
SQRT_HALF = 0.7071067811865476
GAMMA = 1.3
INV_B = 1.0 / B
NEG_BIG = -1e30
DEBUG = False


def _build(prior_mode: str):
    """prior_mode: 'ones' | 'uniform' | 'general'"""
    nc = bacc.Bacc("TRN2", target_bir_lowering=False, debug=False,
                   num_devices=N_CORES)

    a_prev_d = nc.dram_tensor("a_prev", [BS, HALF], F32, kind="ExternalInput")
    x_o_d = nc.dram_tensor("x_o", [BS, IN_DIM], F32, kind="ExternalInput")
    wa_d = nc.dram_tensor("Wa", [HALF, IN_DIM], F32R, kind="ExternalInput")
    w0_d = nc.dram_tensor("W0r", [16, IN_DIM, 128], F32R, kind="ExternalInput")
    w1_d = nc.dram_tensor("W1r", [16, FEAT, 128], F32R, kind="ExternalInput")
    w2_d = nc.dram_tensor("W2r", [16, FEAT, 128], F32R, kind="ExternalInput")
    ga_d = nc.dram_tensor("ga", [IN_DIM], F32, kind="ExternalInput")
    bta_d = nc.dram_tensor("bta", [IN_DIM], F32, kind="ExternalInput")
    gs_d = [nc.dram_tensor(f"g{i}", [2 * FEAT], F32, kind="ExternalInput") for i in range(3)]
    bts_d = [nc.dram_tensor(f"bt{i}", [2 * FEAT], F32, kind="ExternalInput") for i in range(3)]
    if prior_mode == "uniform":
        prow_d = nc.dram_tensor("prior_row", [1, IN_DIM], F32R, kind="ExternalInput")
    elif prior_mode == "general":
        prior_d = nc.dram_tensor("prior", [BS, IN_DIM], F32, kind="ExternalInput")

    d_out = nc.dram_tensor("d_i", [BS, HALF], F32, kind="ExternalOutput")
    a_out = nc.dram_tensor("a_i", [BS, HALF], F32, kind="ExternalOutput")
    mask_out = nc.dram_tensor("mask", [BS, IN_DIM], F32, kind="ExternalOutput")
    pn_out = nc.dram_tensor("prior_next", [BS, IN_DIM], F32, kind="ExternalOutput")
    if DEBUG:
        z_dbg = nc.dram_tensor("z_dbg", [BS, IN_DIM], F32, kind="ExternalOutput")
        st_dbg = nc.dram_tensor("st_dbg", [128, 96], F32, kind="ExternalOutput")

    with tile.TileContext(nc) as tc:
        with (
            tc.tile_pool(name="const", bufs=1) as constp,
            tc.tile_pool(name="wslab", bufs=2) as wslabp,
            tc.tile_pool(name="small", bufs=1) as smallp,
            tc.tile_pool(name="dram", bufs=1, space="DRAM") as dramp,
            tc.tile_pool(name="hps", bufs=2, space="PSUM") as hps,
        ):
            # ---------- constants ----------
            ident = constp.tile([128, 128], F32)
            make_identity(nc, ident[:])
            ones_col_f = constp.tile([128, 1], F32)
            nc.vector.memset(ones_col_f[:], 1.0)
            ones_col = constp.tile([128, 1], F32R)
            nc.scalar.copy(ones_col[:], ones_col_f[:])
            ones_row_f = constp.tile([1, 128], F32)
            nc.vector.memset(ones_row_f[:], 1.0)
            ones_row = constp.tile([1, 128], F32R)
            nc.scalar.copy(ones_row[:], ones_row_f[:])
            kint = constp.tile([128, 16], mybir.dt.int32)
            nc.gpsimd.iota(kint[:], pattern=[[1, 16]], base=1, channel_multiplier=0)
            kf = constp.tile([128, 16], F32)
            nc.vector.tensor_copy(kf[:], kint[:])
            invk = constp.tile([128, 16], F32)
            nc.vector.reciprocal(invk[:], kf[:])

            xTp = tc.alloc_tile_pool(name="xTp", bufs=1, side="right")
            xT = xTp.tile([128, 16, BS], F32R)

            # =========================================================
            # Phase A
            # =========================================================
            with (
                tc.tile_pool(name="phA", bufs=1) as phA,
                tc.tile_pool(name="sparse", bufs=2) as sparsep,
                tc.tile_pool(name="zchunk", bufs=2) as zchp,
                tc.tile_pool(name="zrepp", bufs=1) as zrepp,
                tc.tile_pool(name="mxch", bufs=1) as mxch,
                tc.tile_pool(name="iochunk", bufs=2) as ioch,
                tc.tile_pool(name="zps", bufs=2, space="PSUM") as zps,
                tc.tile_pool(name="stps", bufs=1, space="PSUM") as stps,
                tc.tile_pool(name="trps", bufs=2, space="PSUM") as trps,
            ):
                # transpose a_prev -> a_prevT [128, 4, BS] (f32r)
                a_prevT = phA.tile([128, 4, BS], F32R)
                for c in range(NCH):
                    ach = ioch.tile([128, HALF], F32, tag="ach")
                    nc.sync.dma_start(ach[:], a_prev_d[c * 128:(c + 1) * 128, :])
                    pt = trps.tile([128, HALF], F32, tag="trp")
                    for k in range(4):
                        nc.tensor.transpose(pt[:, k * 128:(k + 1) * 128],
                                            ach[:, k * 128:(k + 1) * 128], ident[:])
                    nc.scalar.copy(a_prevT[:, :, c * 128:(c + 1) * 128],
                                   pt[:].rearrange("p (k x) -> p k x", k=4))

                # colsum of a_prev per feature (reduce over batch on a_prevT)
                colsum_f = smallp.tile([128, 8], F32)
                nc.vector.memset(colsum_f[:], 0.0)
                nc.vector.reduce_sum(
                    colsum_f[:].rearrange("p (f two) -> p f two", two=2)[:, :, 0],
                    a_prevT[:], axis=mybir.AxisListType.X)
                colsum = smallp.tile([128, 8], F32R)
                nc.scalar.copy(colsum[:], colsum_f[:])
                ones_col2_f = smallp.tile([128, 2], F32)
                nc.vector.memset(ones_col2_f[:], 0.0)
                nc.vector.memset(ones_col2_f[:, 0:1], 1.0)
                ones_col2 = smallp.tile([128, 2], F32R)
                nc.scalar.copy(ones_col2[:], ones_col2_f[:])

                s1sb = smallp.tile([128, 16], F32)
                s2sb = smallp.tile([128, 16], F32)
                for ncix in range(4):
                    wsl = wslabp.tile([128, 4, 512], F32R, tag="wslab")
                    nc.sync.dma_start(
                        wsl[:],
                        wa_d[:].rearrange("(kt p) n -> p kt n", p=128)[:, :, ncix * 512:(ncix + 1) * 512])
                    # S1[f-block] = Wa_block.T @ colsum  -> [128, 1] per block
                    s1ps = stps.tile([128, 8], F32, tag="s1ps")
                    for f in range(4):
                        for k in range(4):
                            nc.tensor.matmul(s1ps[:, 2 * f:2 * f + 2],
                                             wsl[:, k, f * 128:(f + 1) * 128],
                                             colsum[:, 2 * k:2 * k + 2],
                                             start=(f == 0 and k == 0),
                                             stop=(f == 3 and k == 3))
                    nc.scalar.copy(s1sb[:, ncix * 4:(ncix + 1) * 4],
                                   s1ps[:].rearrange("p (f two) -> p f two", two=2)[:, :, 0])
                    # S2[f-block] = sum_batch z1^2
                    s2ps = stps.tile([128, 8], F32, tag="s2ps")
                    for c in range(NCH):
                        zp = zps.tile([128, 512], F32, tag="z1ps")
                        for k in range(4):
                            nc.tensor.matmul(zp[:], a_prevT[:, k, c * 128:(c + 1) * 128],
                                             wsl[:, k, :], start=(k == 0), stop=(k == 3))
                        zsq = ioch.tile([128, 512], F32R, tag="zsq")
                        nc.scalar.activation(zsq[:], zp[:], AF.Square)
                        for f in range(4):
                            nc.tensor.matmul(s2ps[:, 2 * f:2 * f + 2],
                                             zsq[:, f * 128:(f + 1) * 128],
                                             ones_col2[:],
                                             start=(c == 0 and f == 0),
                                             stop=(c == NCH - 1 and f == 3))
                    nc.scalar.copy(s2sb[:, ncix * 4:(ncix + 1) * 4],
                                   s2ps[:].rearrange("p (f two) -> p f two", two=2)[:, :, 0])

                # ---- AllReduce BN0 stats ([128, 32]) ----
                st0_in = dramp.tile([128, 32], F32, tag="ar0i")
                st0_out = dramp.tile([128, 32], F32, tag="ar0o")
                nc.sync.dma_start(st0_in[:, 0:16], s1sb[:])
                nc.sync.dma_start(st0_in[:, 16:32], s2sb[:])
                nc.gpsimd.collective_compute(
                    "AllReduce", OP.add, replica_groups=[CORE_IDS],
                    ins=[st0_in[:]], outs=[st0_out[:]])
                gst0 = smallp.tile([128, 32], F32)
                nc.sync.dma_start(gst0[:], st0_out[:])

                # ---- BN0 scale/shift in [128,16], then row-convert via DRAM ----
                mu0 = smallp.tile([128, 16], F32)
                nc.vector.tensor_scalar(mu0[:], gst0[:, 0:16], INV_B, None, op0=OP.mult)
                var0 = smallp.tile([128, 16], F32)
                nc.vector.tensor_scalar(var0[:], gst0[:, 16:32], INV_B, EPS,
                                        op0=OP.mult, op1=OP.add)
                musq0 = smallp.tile([128, 16], F32)
                nc.vector.scalar_tensor_tensor(musq0[:], mu0[:], 0.0, mu0[:],
                                               op0=OP.bypass, op1=OP.mult)
                nc.vector.tensor_sub(var0[:], var0[:], musq0[:])   # var + eps
                rs0 = smallp.tile([128, 16], F32)
                nc.vector.reciprocal(rs0[:], var0[:])
                s0t = smallp.tile([128, 16], F32)
                nc.scalar.sqrt(s0t[:], rs0[:])          # approx rsqrt(var+eps)
                tmp0 = smallp.tile([128, 16], F32)
                nc.vector.scalar_tensor_tensor(tmp0[:], s0t[:], 0.0, s0t[:],
                                               op0=OP.bypass, op1=OP.mult)
                nc.vector.tensor_mul(tmp0[:], tmp0[:], var0[:])
                nc.vector.tensor_scalar(tmp0[:], tmp0[:], -0.5, 1.5, op0=OP.mult, op1=OP.add)
                rsq0 = smallp.tile([128, 16], F32)
                nc.vector.tensor_mul(rsq0[:], s0t[:], tmp0[:])
                gacol = smallp.tile([128, 16], F32)
                nc.sync.dma_start(gacol[:], ga_d[:].rearrange("(m p) -> p m", p=128))
                btacol = smallp.tile([128, 16], F32)
                nc.sync.dma_start(btacol[:], bta_d[:].rearrange("(m p) -> p m", p=128))
                spcol = smallp.tile([128, 16], F32R)
                nc.vector.tensor_mul(spcol[:], gacol[:], rsq0[:])       # scale
                tpcol = smallp.tile([128, 16], F32R)
                nc.vector.tensor_mul(tpcol[:], mu0[:], spcol[:])
                nc.vector.tensor_sub(tpcol[:], btacol[:], tpcol[:])     # shift
                if prior_mode == "uniform":
                    prow = phA.tile([1, IN_DIM], F32R)
                    nc.sync.dma_start(prow[:], prow_d[:])
                    pcol = smallp.tile([128, 16], F32R)
                    nc.sync.dma_start(
                        pcol[:],
                        prow_d[:].rearrange("o (m p) -> (o p) m", p=128))
                    nc.vector.tensor_mul(spcol[:], spcol[:], pcol[:])
                    nc.vector.tensor_mul(tpcol[:], tpcol[:], pcol[:])
                    pbrow = phA.tile([128, IN_DIM], F32)
                    for ncix in range(4):
                        bp2 = zps.tile([128, 512], F32, tag="z1ps")
                        nc.tensor.matmul(bp2[:], ones_row[:],
                                         prow[:, ncix * 512:(ncix + 1) * 512],
                                         start=True, stop=True)
                        nc.scalar.copy(pbrow[:, ncix * 512:(ncix + 1) * 512], bp2[:])

                if DEBUG:
                    nc.sync.dma_start(st_dbg[:, 0:16], s1sb[:])
                    nc.sync.dma_start(st_dbg[:, 16:32], s2sb[:])
                    nc.sync.dma_start(st_dbg[:, 32:64], gst0[:])
                    stf1 = smallp.tile([128, 16], F32)
                    nc.vector.tensor_copy(stf1[:], spcol[:])
                    nc.sync.dma_start(st_dbg[:, 64:80], stf1[:])
                    stf2 = smallp.tile([128, 16], F32)
                    nc.vector.tensor_copy(stf2[:], tpcol[:])
                    nc.sync.dma_start(st_dbg[:, 80:96], stf2[:])
                # col -> row conversion via DRAM roundtrip (strided reload)
                sp_dram = dramp.tile([128, 16], F32R, tag="spd")
                tp_dram = dramp.tile([128, 16], F32R, tag="tpd")
                nc.sync.dma_start(sp_dram[:], spcol[:])
                nc.sync.dma_start(tp_dram[:], tpcol[:])
                sprow = phA.tile([1, IN_DIM], F32R)
                nc.sync.dma_start(
                    sprow[:].rearrange("o (m p) -> o m p", m=16),
                    sp_dram[:].rearrange("p m -> m p"))
                tprow = phA.tile([1, IN_DIM], F32R)
                nc.sync.dma_start(
                    tprow[:].rearrange("o (m p) -> o m p", m=16),
                    tp_dram[:].rearrange("p m -> m p"))

                # broadcast scale, build Wa'' = Wa * scale_cols
                spbp = tc.alloc_tile_pool(name="spbp", bufs=1)
                spb = spbp.tile([128, IN_DIM], F32)
                for ncix in range(4):
                    bp = zps.tile([128, 512], F32, tag="z1ps")
                    nc.tensor.matmul(bp[:], ones_row[:], sprow[:, ncix * 512:(ncix + 1) * 512],
                                     start=True, stop=True)
                    nc.scalar.copy(spb[:, ncix * 512:(ncix + 1) * 512], bp[:])
                wa2 = phA.tile([128, 4, IN_DIM], F32R)
                for k in range(4):
                    wsl = wslabp.tile([128, 4, 512], F32R, tag="wslab")
                    nc.sync.dma_start(
                        wsl[:],
                        wa_d[:].rearrange("(kt p) n -> kt p n", p=128)[k].rearrange("p (a n) -> p a n", a=4))
                    for a in range(4):
                        nc.vector.tensor_mul(wa2[:, k, a * 512:(a + 1) * 512],
                                             wsl[:, a, :], spb[:, a * 512:(a + 1) * 512])
                spbp.release()

                # ---- pass 2 + sparsemax + x^T, per chunk ----
                for c in range(NCH):
                    zch = zchp.tile([128, IN_DIM], F32, tag="z")
                    for ncix in range(4):
                        zp = zps.tile([128, 512], F32, tag="z1ps")
                        for k in range(4):
                            nc.tensor.matmul(zp[:], a_prevT[:, k, c * 128:(c + 1) * 128],
                                             wa2[:, k, ncix * 512:(ncix + 1) * 512],
                                             start=(k == 0), stop=False)
                        nc.tensor.matmul(zp[:], ones_row[:],
                                         tprow[:, ncix * 512:(ncix + 1) * 512],
                                         start=False, stop=True)
                        nc.scalar.copy(zch[:, ncix * 512:(ncix + 1) * 512], zp[:])
                    if DEBUG:
                        nc.sync.dma_start(z_dbg[c * 128:(c + 1) * 128, :], zch[:])
                    if prior_mode == "general":
                        pch = mxch.tile([128, IN_DIM], F32, tag="pch")
                        nc.sync.dma_start(pch[:], prior_d[c * 128:(c + 1) * 128, :])
                        nc.vector.tensor_mul(zch[:], zch[:], pch[:])

                    # --- sparsemax closed form on top-16 ---
                    t16 = sparsep.tile([128, 16], F32, tag="t16")
                    nc.vector.max(t16[:, 0:8], zch[:])
                    zrep = zrepp.tile([128, IN_DIM], F32, tag="zrep")
                    nc.vector.match_replace(zrep[:], in_to_replace=t16[:, 0:8],
                                            in_values=zch[:], imm_value=NEG_BIG)
                    nc.vector.max(t16[:, 8:16], zrep[:])
                    c2 = sparsep.tile([128, 16], F32, tag="c2")
                    nc.vector.tensor_copy(c2[:, 0:1], t16[:, 0:1])
                    nc.vector.tensor_add(c2[:, 1:16], t16[:, 1:16], t16[:, 0:15])
                    c3 = sparsep.tile([128, 16], F32, tag="c3")
                    nc.vector.tensor_copy(c3[:, 0:2], c2[:, 0:2])
                    nc.vector.tensor_add(c3[:, 2:16], c2[:, 2:16], c2[:, 0:14])
                    nc.vector.tensor_copy(c2[:, 0:4], c3[:, 0:4])
                    nc.vector.tensor_add(c2[:, 4:16], c3[:, 4:16], c3[:, 0:12])
                    nc.vector.tensor_copy(c3[:, 0:8], c2[:, 0:8])
                    nc.vector.tensor_add(c3[:, 8:16], c2[:, 8:16], c2[:, 0:8])
                    nc.vector.tensor_scalar(c3[:], c3[:], -1.0, None, op0=OP.add)
                    nc.vector.tensor_mul(c3[:], c3[:], invk[:])
                    ntau = sparsep.tile([128, 1], F32, tag="ntau")
                    nc.vector.tensor_reduce(ntau[:], c3[:], axis=mybir.AxisListType.X,
                                            op=OP.max, negate=True)
                    # mask = relu(z - tau)
                    mch = mxch.tile([128, IN_DIM], F32, tag="mask")
                    nc.scalar.activation(mch[:], zch[:], AF.Relu, bias=ntau[:])
                    nc.sync.dma_start(mask_out[c * 128:(c + 1) * 128, :], mch[:])
                    # x = x_o * mask
                    xch = mxch.tile([128, IN_DIM], F32, tag="xch")
                    nc.sync.dma_start(xch[:], x_o_d[c * 128:(c + 1) * 128, :])
                    nc.vector.tensor_mul(xch[:], xch[:], mch[:])
                    # prior_next = (gamma - mask) * prior (in-place over mask tile)
                    nc.vector.tensor_scalar(mch[:], mch[:], -1.0, GAMMA,
                                            op0=OP.mult, op1=OP.add)
                    if prior_mode == "general":
                        nc.vector.tensor_mul(mch[:], mch[:], pch[:])
                    elif prior_mode == "uniform":
                        nc.vector.tensor_mul(mch[:], mch[:], pbrow[:])
                    nc.sync.dma_start(pn_out[c * 128:(c + 1) * 128, :], mch[:])
                    # transpose x chunk into xT
                    for g in range(4):
                        pt = trps.tile([128, 512], F32, tag="trp")
                        for j in range(4):
                            nc.tensor.transpose(pt[:, j * 128:(j + 1) * 128],
                                                xch[:, (4 * g + j) * 128:(4 * g + j + 1) * 128],
                                                ident[:])
                        nc.scalar.copy(
                            xT[:, 4 * g:4 * g + 4, c * 128:(c + 1) * 128],
                            pt[:].rearrange("p (k x) -> p k x", k=4))

            # =========================================================
            # Phase B
            # =========================================================
            with tc.tile_pool(name="ioch2", bufs=1) as ioch2:

                def glu_stage(sidx, rhsT, k_tiles, w_dram, g_dram, bt_dram, outp):
                    with tc.tile_pool(name=f"hT{sidx}", bufs=1) as hTp:
                        hT = hTp.tile([128, 16, BS], F32)
                        s1a = smallp.tile([128, 16], F32, tag=f"s1a{sidx}")
                        s1b = smallp.tile([128, 16], F32, tag=f"s1b{sidx}")
                        s2a = smallp.tile([128, 16], F32, tag=f"s2a{sidx}")
                        s2b = smallp.tile([128, 16], F32, tag=f"s2b{sidx}")
                        for mo in range(16):
                            wsl = wslabp.tile([128, k_tiles, 128], F32R, tag="wslab")
                            nc.sync.dma_start(
                                wsl[:], w_dram[mo].rearrange("(kt p) m -> p kt m", p=128))
                            for h in range(2):
                                hp = hps.tile([128, 512], F32, tag="hps")
                                for k in range(k_tiles):
                                    nc.tensor.matmul(hp[:], wsl[:, k, :],
                                                     rhsT[:, k, h * 512:(h + 1) * 512],
                                                     start=(k == 0), stop=(k == k_tiles - 1))
                                s1 = (s1a if h == 0 else s1b)
                                nc.scalar.activation(hT[:, mo, h * 512:(h + 1) * 512], hp[:],
                                                     AF.Copy, accum_out=s1[:, mo:mo + 1])
                                sq = ioch2.tile([128, 512], F32, tag="sq")
                                s2 = (s2a if h == 0 else s2b)
                                nc.vector.scalar_tensor_tensor(
                                    sq[:], hT[:, mo, h * 512:(h + 1) * 512], 0.0,
                                    hT[:, mo, h * 512:(h + 1) * 512],
                                    op0=OP.bypass, op1=OP.mult, accum_out=s2[:, mo:mo + 1])
                        stats = smallp.tile([128, 32], F32, tag=f"st{sidx}")
                        nc.vector.tensor_add(stats[:, 0:16], s1a[:], s1b[:])
                        nc.vector.tensor_add(stats[:, 16:32], s2a[:], s2b[:])
                        bin_ = dramp.tile([128, 32], F32, tag=f"arin{sidx}")
                        bout = dramp.tile([128, 32], F32, tag=f"arout{sidx}")
                        nc.sync.dma_start(bin_[:], stats[:])
                        nc.gpsimd.collective_compute(
                            "AllReduce", OP.add, replica_groups=[CORE_IDS],
                            ins=[bin_[:]], outs=[bout[:]])
                        gst = smallp.tile([128, 32], F32, tag=f"gst{sidx}")
                        nc.sync.dma_start(gst[:], bout[:])
                        mu = smallp.tile([128, 16], F32, tag=f"mu{sidx}")
                        nc.vector.tensor_scalar(mu[:], gst[:, 0:16], INV_B, None, op0=OP.mult)
                        var = smallp.tile([128, 16], F32, tag=f"var{sidx}")
                        nc.vector.tensor_scalar(var[:], gst[:, 16:32], INV_B, EPS,
                                                op0=OP.mult, op1=OP.add)
                        musq = smallp.tile([128, 16], F32, tag=f"musq{sidx}")
                        nc.vector.scalar_tensor_tensor(musq[:], mu[:], 0.0, mu[:],
                                                       op0=OP.bypass, op1=OP.mult)
                        nc.vector.tensor_sub(var[:], var[:], musq[:])
                        rsp = smallp.tile([128, 16], F32, tag=f"rsp{sidx}")
                        nc.vector.reciprocal(rsp[:], var[:])
                        sq0 = smallp.tile([128, 16], F32, tag=f"sq0{sidx}")
                        nc.scalar.sqrt(sq0[:], rsp[:])
                        tmp = smallp.tile([128, 16], F32, tag=f"tmp{sidx}")
                        nc.vector.scalar_tensor_tensor(tmp[:], sq0[:], 0.0, sq0[:],
                                                       op0=OP.bypass, op1=OP.mult)
                        nc.vector.tensor_mul(tmp[:], tmp[:], var[:])
                        nc.vector.tensor_scalar(tmp[:], tmp[:], -0.5, 1.5,
                                                op0=OP.mult, op1=OP.add)
                        rs = smallp.tile([128, 16], F32, tag=f"rs{sidx}")
                        nc.vector.tensor_mul(rs[:], sq0[:], tmp[:])
                        gsb = smallp.tile([128, 16], F32, tag=f"gsb{sidx}")
                        nc.sync.dma_start(gsb[:], g_dram[:].rearrange("(m p) -> p m", p=128))
                        btsb = smallp.tile([128, 16], F32, tag=f"btsb{sidx}")
                        nc.sync.dma_start(btsb[:], bt_dram[:].rearrange("(m p) -> p m", p=128))
                        scl = smallp.tile([128, 16], F32, tag=f"scl{sidx}")
                        nc.vector.tensor_mul(scl[:], gsb[:], rs[:])
                        shf = smallp.tile([128, 16], F32, tag=f"shf{sidx}")
                        nc.vector.tensor_mul(shf[:], mu[:], scl[:])
                        nc.vector.tensor_sub(shf[:], btsb[:], shf[:])
                        gout = outp.tile([128, 8, BS], F32R)
                        for a in range(8):
                            for h in range(2):
                                sl = slice(h * 512, (h + 1) * 512)
                                sg = ioch2.tile([128, 512], F32, tag="sg")
                                nc.scalar.activation(sg[:], hT[:, a + 8, sl], AF.Sigmoid,
                                                     bias=shf[:, a + 8:a + 9],
                                                     scale=scl[:, a + 8:a + 9])
                                na = ioch2.tile([128, 512], F32, tag="na")
                                nc.scalar.activation(na[:], hT[:, a, sl], AF.Identity,
                                                     bias=shf[:, a:a + 1],
                                                     scale=scl[:, a:a + 1])
                                nc.vector.tensor_mul(gout[:, a, sl], na[:], sg[:])
                        return gout

                x0Tp = tc.alloc_tile_pool(name="x0Tp", bufs=1)
                x0T = glu_stage(1, xT, 16, w0_d, gs_d[0], bts_d[0], x0Tp)
                xTp.release()
                g1p = tc.alloc_tile_pool(name="g1p", bufs=1)
                g1 = glu_stage(2, x0T, 8, w1_d, gs_d[1], bts_d[1], g1p)
                y1Tp = tc.alloc_tile_pool(name="y1Tp", bufs=1, side="right")
                y1T = y1Tp.tile([128, 8, BS], F32R)
                for a in range(8):
                    nc.vector.tensor_add(y1T[:, a, :], g1[:, a, :], x0T[:, a, :])
                g1p.release()
                x0Tp.release()
                g2p = tc.alloc_tile_pool(name="g2p", bufs=1, side="right")
                g2 = glu_stage(3, y1T, 8, w2_d, gs_d[2], bts_d[2], g2p)
                # x2 = 0.5*y1 + G   (sqrt(1/2) folded into g2/bt2 A-half + W2)
                x2p = tc.alloc_tile_pool(name="x2p", bufs=1)
                x2T = x2p.tile([128, 8, BS], F32)
                for a in range(8):
                    nc.vector.scalar_tensor_tensor(
                        x2T[:, a, :], y1T[:, a, :], 0.5, g2[:, a, :],
                        op0=OP.mult, op1=OP.add)
                g2p.release()
                y1Tp.release()
                with tc.tile_pool(name="ops", bufs=2, space="PSUM") as ops_ps:
                    for c in range(NCH):
                        for part, out_d, func in ((0, d_out, AF.Relu), (1, a_out, AF.Copy)):
                            pt = ops_ps.tile([128, 512], F32, tag="ops")
                            for f in range(4):
                                nc.tensor.transpose(
                                    pt[:, f * 128:(f + 1) * 128],
                                    x2T[:, part * 4 + f, c * 128:(c + 1) * 128], ident[:])
                            osb = ioch2.tile([128, 512], F32, tag="osb")
                            if func == AF.Relu:
                                nc.scalar.activation(osb[:], pt[:], AF.Relu)
                            else:
                                nc.scalar.copy(osb[:], pt[:])
                            nc.sync.dma_start(out_d[c * 128:(c + 1) * 128, :], osb[:])
                x2p.release()
    nc.finalize()
    return nc


_NC_CACHE = {}


def kernel(**inputs):
    inp = {k: np.ascontiguousarray(np.asarray(v, dtype=np.float32))
           for k, v in inputs.items()}
    prior = inp["prior"]
    if np.all(prior == 1.0):
        prior_mode = "ones"
    elif np.all(prior == prior[0:1, :]):
        prior_mode = "uniform"
    else:
        prior_mode = "general"

    key = (prior_mode, DEBUG)
    if key not in _NC_CACHE:
        _NC_CACHE[key] = _build(prior_mode)
    nc = _NC_CACHE[key]

    W0r = np.ascontiguousarray(inp["W0"].reshape(IN_DIM, 16, 128).transpose(1, 0, 2))
    W1r = np.ascontiguousarray(inp["W1"].reshape(FEAT, 16, 128).transpose(1, 0, 2))
    W2r = np.ascontiguousarray((inp["W2"] * SQRT_HALF).reshape(FEAT, 16, 128).transpose(1, 0, 2))
    g2 = inp["g2"].copy(); g2[:FEAT] *= SQRT_HALF
    bt2 = inp["bt2"].copy(); bt2[:FEAT] *= SQRT_HALF

    common = dict(
        Wa=inp["Wa"], W0r=W0r, W1r=W1r, W2r=W2r,
        ga=inp["ga"], bta=inp["bta"],
        g0=inp["g0"], bt0=inp["bt0"],
        g1=inp["g1"], bt1=inp["bt1"],
        g2=g2, bt2=bt2,
    )
    if prior_mode == "uniform":
        common["prior_row"] = np.ascontiguousarray(prior[0:1, :])

    in_maps = []
    for c in range(N_CORES):
        m = dict(common)
        m["a_prev"] = inp["a_prev"][c * BS:(c + 1) * BS]
        m["x_o"] = inp["x_o"][c * BS:(c + 1) * BS]
        if prior_mode == "general":
            m["prior"] = prior[c * BS:(c + 1) * BS]
        in_maps.append(m)

    res = run_bass_kernel_spmd(nc, in_maps, CORE_IDS)
    d_i = np.concatenate([res.results[c]["d_i"] for c in range(N_CORES)], axis=0)
    a_i = np.concatenate([res.results[c]["a_i"] for c in range(N_CORES)], axis=0)
    mask = np.concatenate([res.results[c]["mask"] for c in range(N_CORES)], axis=0)
    pn = np.concatenate([res.results[c]["prior_next"] for c in range(N_CORES)], axis=0)
    return (d_i, a_i, mask, pn)
